# revision 1
# baseline (speedup 1.0000x reference)
"""Trainium2 Bass kernel for nn_HamiltonianDynamics.

Math: with q = state[:, :8], p = state[:, 8:], every MLP evaluation in the
reference operates on per-batch means of q/p. Adding a constant c to every
element of a [8,256,256] block shifts its mean by exactly c, so the whole
leapfrog chain (g1, g2, g3), the casimir correction and the global norm are
computable from just per-batch sums and sums of squares:

  out = (state + off[b, half]) * scale
  off_q[b] = dt*g2[b,1]/Nq,  off_p[b] = -0.5*dt*(g1[b,0]+g3[b,0])/Nq
  norm^2   = sum_b,h ( ssq[b,h] + 2*off[b,h]*sum[b,h] + Nq*off[b,h]^2 )
  scale    = 1 - 0.1*err/(norm+1e-10)

One fused SPMD kernel: reduce pass (shard stays resident in SBUF), tiny
AllGather of [1,16] partial stats, on-device MLP gradient chain (batch on the
free axis, features on partitions), then in-place transform + store.

Engine-AP constraint: compute-engine APs must start at partition 0 (quarter
boundaries), so all per-batch row vectors live in separate [1,nb] tiles and
the 2-feature input layers are done as two accumulated K=1 matmuls.
"""

import numpy as np

NCORES = 8
B, CH, H, W = 32, 16, 256, 256
BPC = B // NCORES          # batches per core
NTILES = BPC * 2           # (batch, half) tiles per core
P = 128
FREE = (CH // 2) * H * W // P   # 4096
NQ = (CH // 2) * H * W          # 524288

_CACHE: dict = {}


def build_nc(ncores=NCORES, bpc=BPC, free=FREE, nchunks=4, debug_out=True,
             dma_mix=False):
    import concourse.bass as bass
    import concourse.bacc as bacc
    import concourse.tile as tile
    import concourse.mybir as mybir
    from contextlib import ExitStack

    f32 = mybir.dt.float32
    AL = mybir.AluOpType
    AF = mybir.ActivationFunctionType
    AX = mybir.AxisListType

    ntiles = bpc * 2
    nb = ncores * bpc
    nq = float(P * free)
    csz = free // nchunks

    nc = bacc.Bacc("TRN2", target_bir_lowering=False, debug=False,
                   num_devices=ncores)

    def din(name, shape):
        return nc.dram_tensor(name, shape, f32, kind="ExternalInput").ap()

    x = din("x", [ntiles, P, free])
    w1a = din("w1a", [1, 128]);  w1b = din("w1b", [1, 128])
    b1 = din("b1", [128, 1])
    w2 = din("w2", [128, 128]);  b2 = din("b2", [128, 1])
    w3 = din("w3", [128, 64]);   b3 = din("b3", [64, 1])
    w4 = din("w4", [64, 1]);     w4n = din("w4n", [64, 1])
    w1t = din("w1t", [128, 2]);  w2t = din("w2t", [128, 128])
    w3t = din("w3t", [64, 128])
    cw1a = din("cw1a", [1, 64]); cw1b = din("cw1b", [1, 64])
    cb1 = din("cb1", [64, 1])
    cw2 = din("cw2", [64, 32]);  cb2 = din("cb2", [32, 1])
    cw3 = din("cw3", [32, 4])
    sel = din("sel", [nb, bpc])          # per-core one-hot batch selector
    aux = din("aux", [1, 2])             # [-0.5*dt/Nq, dt/Nq]
    y = nc.dram_tensor("y", [ntiles, P, free], f32, kind="ExternalOutput").ap()
    if debug_out:
        dbg = nc.dram_tensor("dbg", [8, nb], f32, kind="ExternalOutput").ap()

    with tile.TileContext(nc) as tc, ExitStack() as ctx:
        xpool = ctx.enter_context(tc.tile_pool(name="xp", bufs=1))
        wpool = ctx.enter_context(tc.tile_pool(name="wp", bufs=1))
        scr = ctx.enter_context(tc.tile_pool(name="scr", bufs=2))
        ch = ctx.enter_context(tc.tile_pool(name="ch", bufs=2))
        keep = ctx.enter_context(tc.tile_pool(name="keep", bufs=1))
        psum = ctx.enter_context(tc.tile_pool(name="ps", bufs=4, space="PSUM"))
        dram = ctx.enter_context(tc.tile_pool(name="dr", bufs=1, space="DRAM"))

        ones_col = wpool.tile([128, 1], f32)     # lhsT for partition sums
        nc.vector.memset(ones_col[:], 1.0)
        ones_bc = wpool.tile([1, 128], f32)      # lhsT for partition broadcast
        nc.vector.memset(ones_bc[:], 1.0)

        # ---- phase A: load shard, per-(batch,half) sum and sumsq ----
        # per-tile stats [128,2] (col0=sum, col1=ssq); partition-summed into
        # part_ps columns 2t..2t+1 via 8 independent PE matmuls
        part_ps = psum.tile([1, 4 * bpc], f32, tag="ps")
        xts = []
        for t in range(ntiles):
            xt = xpool.tile([P, free], f32, tag=f"x{t}")
            for c in range(nchunks):
                eng = nc.gpsimd if (dma_mix and (t * nchunks + c) % 2) else nc.sync
                eng.dma_start(xt[:, c * csz:(c + 1) * csz],
                              x[t][:, c * csz:(c + 1) * csz])
            xts.append(xt)
            # per-chunk partial stats, accumulated across chunks in PSUM so
            # the reduction tail after the last chunk lands is ~one chunk
            for c in range(nchunks):
                xc = xt[:, c * csz:(c + 1) * csz]
                st = keep.tile([128, 2], f32, tag=f"st{t}_{c}")
                nc.vector.tensor_reduce(st[:, 0:1], xc, axis=AX.X, op=AL.add)
                sq = scr.tile([P, csz], f32, tag="sq")
                nc.scalar.activation(sq[:], xc, AF.Square,
                                     accum_out=st[:, 1:2])
                nc.tensor.matmul(part_ps[0:1, 2 * t:2 * t + 2], ones_col[:],
                                 st[:], start=(c == 0), stop=(c == nchunks - 1))

        # ---- weights / constants to SBUF ----
        def wload(ap, shape):
            t = wpool.tile(shape, f32, tag=ap.tensor.name)
            nc.gpsimd.dma_start(t[:], ap)
            return t

        w1a_sb = wload(w1a, [1, 128]); w1b_sb = wload(w1b, [1, 128])
        w2_sb = wload(w2, [128, 128]); w3_sb = wload(w3, [128, 64])
        b1_sb = wload(b1, [128, 1]); b2_sb = wload(b2, [128, 1])
        b3_sb = wload(b3, [64, 1])
        w4_sb = wload(w4, [64, 1]); w4n_sb = wload(w4n, [64, 1])
        w1t_sb = wload(w1t, [128, 2]); w2t_sb = wload(w2t, [128, 128])
        w3t_sb = wload(w3t, [64, 128])
        cw1a_sb = wload(cw1a, [1, 64]); cw1b_sb = wload(cw1b, [1, 64])
        cb1_sb = wload(cb1, [64, 1])
        cw2_sb = wload(cw2, [64, 32]); cb2_sb = wload(cb2, [32, 1])
        cw3_sb = wload(cw3, [32, 4])
        sel_sb = wload(sel, [nb, bpc])
        aux_sb = wload(aux, [1, 2])

        # ---- phase B: relayout to s-major + AllGather ----
        # part_ps col 2*(2*bl+h)+s  ->  part_sb col s_major = s*bpc+bl,
        # s in {0:sum_q, 1:sum_p, 2:ssq_q, 3:ssq_p}
        part_sb = keep.tile([1, 4 * bpc], f32)
        off_of_s = [0, 2, 1, 3]  # (h,stat): s0=(0,sum)->4bl+0, s1=(1,sum)->4bl+2,
        #                          s2=(0,ssq)->4bl+1, s3=(1,ssq)->4bl+3
        for s in range(4):
            nc.vector.tensor_copy(
                part_sb[0:1, s * bpc:(s + 1) * bpc],
                part_ps[0:1, off_of_s[s]:4 * bpc:4])

        cc_in = dram.tile([1, 4 * bpc], f32)
        cc_out = dram.tile([ncores, 4 * bpc], f32)
        nc.sync.dma_start(cc_in[:], part_sb[:])
        nc.gpsimd.collective_compute(
            "AllGather", AL.bypass,
            replica_groups=[list(range(ncores))],
            ins=[cc_in[:].opt()], outs=[cc_out[:].opt()])

        # Rj: j=0 sum_q[b], 1 sum_p[b], 2 ssq_q[b], 3 ssq_p[b]; each [1,nb]
        Rt = []
        for j in range(4):
            rj = keep.tile([1, nb], f32, tag=f"R{j}")
            nc.sync.dma_start(rj[:], cc_out[:, j * bpc:(j + 1) * bpc])
            Rt.append(rj)

        # ---- phase C: scalar chain (features on partitions, batch on free) ----
        def gH(mq, mp, want):
            """grad of sum(ham MLP) wrt (mq, mp): [1,nb] psum, row `want`."""
            p1 = psum.tile([128, nb], f32, tag="ps")
            nc.tensor.matmul(p1[:], w1a_sb[:], mq[:], start=True, stop=False)
            nc.tensor.matmul(p1[:], w1b_sb[:], mp[:], start=False, stop=True)
            h1 = ch.tile([128, nb], f32, tag="h1")
            nc.scalar.activation(h1[:], p1[:], AF.Tanh, bias=b1_sb[:])
            p2 = psum.tile([128, nb], f32, tag="ps")
            nc.tensor.matmul(p2[:], w2_sb[:], h1[:], start=True, stop=True)
            h2 = ch.tile([128, nb], f32, tag="h2")
            nc.scalar.activation(h2[:], p2[:], AF.Tanh, bias=b2_sb[:])
            p3 = psum.tile([64, nb], f32, tag="ps")
            nc.tensor.matmul(p3[:], w3_sb[:], h2[:], start=True, stop=True)
            h3 = ch.tile([64, nb], f32, tag="h3")
            nc.scalar.activation(h3[:], p3[:], AF.Tanh, bias=b3_sb[:])
            # d3 = (1 - h3^2) * W4  ==  (h3^2) * (-W4) + W4
            d3 = ch.tile([64, nb], f32, tag="d3")
            nc.vector.tensor_tensor(d3[:], h3[:], h3[:], op=AL.mult)
            nc.vector.tensor_scalar(d3[:], d3[:], scalar1=w4n_sb[:],
                                    scalar2=w4_sb[:], op0=AL.mult, op1=AL.add)
            pd2 = psum.tile([128, nb], f32, tag="ps")
            nc.tensor.matmul(pd2[:], w3t_sb[:], d3[:], start=True, stop=True)
            t2 = ch.tile([128, nb], f32, tag="t2")
            nc.vector.tensor_tensor(t2[:], h2[:], h2[:], op=AL.mult)
            nc.vector.tensor_scalar(t2[:], t2[:], scalar1=-1.0, scalar2=1.0,
                                    op0=AL.mult, op1=AL.add)
            d2 = ch.tile([128, nb], f32, tag="d2")
            nc.vector.tensor_tensor(d2[:], t2[:], pd2[:], op=AL.mult)
            pd1 = psum.tile([128, nb], f32, tag="ps")
            nc.tensor.matmul(pd1[:], w2t_sb[:], d2[:], start=True, stop=True)
            t1 = ch.tile([128, nb], f32, tag="t1")
            nc.vector.tensor_tensor(t1[:], h1[:], h1[:], op=AL.mult)
            nc.vector.tensor_scalar(t1[:], t1[:], scalar1=-1.0, scalar2=1.0,
                                    op0=AL.mult, op1=AL.add)
            d1 = ch.tile([128, nb], f32, tag="d1")
            nc.vector.tensor_tensor(d1[:], t1[:], pd1[:], op=AL.mult)
            pg = psum.tile([1, nb], f32, tag="ps")
            col = 0 if want == "q" else 1
            nc.tensor.matmul(pg[:], w1t_sb[:, col:col + 1], d1[:],
                             start=True, stop=True)
            return pg

        def cas_h2(mq, mp, tag):
            """second hidden layer of casimir MLP -> [32,nb] sbuf."""
            q1 = psum.tile([64, nb], f32, tag="ps")
            nc.tensor.matmul(q1[:], cw1a_sb[:], mq[:], start=True, stop=False)
            nc.tensor.matmul(q1[:], cw1b_sb[:], mp[:], start=False, stop=True)
            g1 = ch.tile([64, nb], f32, tag="cg1")
            nc.scalar.activation(g1[:], q1[:], AF.Tanh, bias=cb1_sb[:])
            q2 = psum.tile([32, nb], f32, tag="ps")
            nc.tensor.matmul(q2[:], cw2_sb[:], g1[:], start=True, stop=True)
            g2 = ch.tile([32, nb], f32, tag=tag)
            nc.scalar.activation(g2[:], q2[:], AF.Tanh, bias=cb2_sb[:])
            return g2

        mq = keep.tile([1, nb], f32)
        nc.vector.tensor_scalar(mq[:], Rt[0][:], scalar1=1.0 / nq,
                                scalar2=None, op0=AL.mult)
        mp = keep.tile([1, nb], f32)
        nc.vector.tensor_scalar(mp[:], Rt[1][:], scalar1=1.0 / nq,
                                scalar2=None, op0=AL.mult)
        pg1 = gH(mq, mp, "q")
        o1 = keep.tile([1, nb], f32)
        nc.vector.tensor_scalar(o1[:], pg1[:], scalar1=aux_sb[0:1, 0:1],
                                scalar2=None, op0=AL.mult)
        mp2 = keep.tile([1, nb], f32)
        nc.vector.tensor_tensor(mp2[:], mp[:], o1[:], op=AL.add)
        pg2 = gH(mq, mp2, "p")
        offq = keep.tile([1, nb], f32)
        nc.vector.tensor_scalar(offq[:], pg2[:], scalar1=aux_sb[0:1, 1:2],
                                scalar2=None, op0=AL.mult)
        mq3 = keep.tile([1, nb], f32)
        nc.vector.tensor_tensor(mq3[:], mq[:], offq[:], op=AL.add)
        pg3 = gH(mq3, mp2, "q")
        o3 = keep.tile([1, nb], f32)
        nc.vector.tensor_scalar(o3[:], pg3[:], scalar1=aux_sb[0:1, 0:1],
                                scalar2=None, op0=AL.mult)
        offp = keep.tile([1, nb], f32)
        nc.vector.tensor_tensor(offp[:], o1[:], o3[:], op=AL.add)
        mpn = keep.tile([1, nb], f32)
        nc.vector.tensor_tensor(mpn[:], mp[:], offp[:], op=AL.add)

        # selection on UNSCALED offsets (overlaps the casimir/norm path);
        # scale is applied to the tiny selected vectors at the end
        colq = keep.tile([nb, 1], f32)
        nc.sync.dma_start(colq[:], offq[:])
        colp = keep.tile([nb, 1], f32)
        nc.sync.dma_start(colp[:], offp[:])
        pselq = psum.tile([1, bpc], f32, tag="ps")
        nc.tensor.matmul(pselq[:], colq[:], sel_sb[:], start=True, stop=True)
        pselp = psum.tile([1, bpc], f32, tag="ps")
        nc.tensor.matmul(pselp[:], colp[:], sel_sb[:], start=True, stop=True)

        # casimir err: sum over (4, nb) of cW3^T @ (h2_new - h2_old)
        g2o = cas_h2(mq, mp, "g2o")
        g2n = cas_h2(mq3, mpn, "g2n")
        dh = ch.tile([32, nb], f32, tag="dh")
        nc.vector.tensor_tensor(dh[:], g2n[:], g2o[:], op=AL.subtract)
        qd = psum.tile([4, nb], f32, tag="ps")
        nc.tensor.matmul(qd[:], cw3_sb[:], dh[:], start=True, stop=True)
        dsum = keep.tile([4, 1], f32)
        nc.vector.tensor_reduce(dsum[:], qd[:], axis=AX.X, op=AL.add)
        pe = psum.tile([1, 1], f32, tag="ps")
        nc.tensor.matmul(pe[:], ones_col[0:4, 0:1], dsum[:], start=True, stop=True)
        err = keep.tile([1, 1], f32)
        nc.vector.tensor_copy(err[:], pe[:])

        # norm^2 per batch, then total
        n2 = keep.tile([1, nb], f32)
        u1 = ch.tile([1, nb], f32, tag="u1")
        nc.vector.tensor_tensor(u1[:], offq[:], Rt[0][:], op=AL.mult)
        nc.vector.tensor_scalar(u1[:], u1[:], scalar1=2.0, scalar2=None, op0=AL.mult)
        u2 = ch.tile([1, nb], f32, tag="u2")
        nc.vector.tensor_tensor(u2[:], offq[:], offq[:], op=AL.mult)
        nc.vector.tensor_scalar(u2[:], u2[:], scalar1=nq, scalar2=None, op0=AL.mult)
        nc.vector.tensor_tensor(n2[:], Rt[2][:], u1[:], op=AL.add)
        nc.vector.tensor_tensor(n2[:], n2[:], u2[:], op=AL.add)
        v1 = ch.tile([1, nb], f32, tag="v1")
        nc.vector.tensor_tensor(v1[:], offp[:], Rt[1][:], op=AL.mult)
        nc.vector.tensor_scalar(v1[:], v1[:], scalar1=2.0, scalar2=None, op0=AL.mult)
        v2 = ch.tile([1, nb], f32, tag="v2")
        nc.vector.tensor_tensor(v2[:], offp[:], offp[:], op=AL.mult)
        nc.vector.tensor_scalar(v2[:], v2[:], scalar1=nq, scalar2=None, op0=AL.mult)
        nc.vector.tensor_tensor(n2[:], n2[:], Rt[3][:], op=AL.add)
        nc.vector.tensor_tensor(n2[:], n2[:], v1[:], op=AL.add)
        nc.vector.tensor_tensor(n2[:], n2[:], v2[:], op=AL.add)
        nsum = keep.tile([1, 1], f32)
        nc.vector.tensor_reduce(nsum[:], n2[:], axis=AX.X, op=AL.add)
        nrm = keep.tile([1, 1], f32)
        nc.scalar.sqrt(nrm[:], nsum[:])
        den = keep.tile([1, 1], f32)
        nc.vector.tensor_scalar(den[:], nrm[:], scalar1=1e-10, scalar2=None,
                                op0=AL.add)
        rec = keep.tile([1, 1], f32)
        nc.vector.reciprocal(rec[:], den[:])
        scv = keep.tile([1, 1], f32)
        nc.vector.tensor_tensor(scv[:], err[:], rec[:], op=AL.mult)
        # scale = 1 - (0.1/(4*nb)) * errsum / (norm+1e-10)
        nc.vector.tensor_scalar(scv[:], scv[:], scalar1=-0.1 / (4.0 * nb),
                                scalar2=1.0, op0=AL.mult, op1=AL.add)

        if debug_out:
            nc.gpsimd.dma_start(dbg[0:1, :], offq[:])
            nc.gpsimd.dma_start(dbg[1:2, :], offp[:])
            for j in range(4):
                nc.gpsimd.dma_start(dbg[2 + j:3 + j, :], Rt[j][:])
            nc.gpsimd.dma_start(dbg[6:7, 0:1], scv[:])
            nc.gpsimd.dma_start(dbg[7:8, 0:1], err[:])

        # ---- phase D: scale selected offsets + partition broadcast ----
        Bv = keep.tile([1, 2 * bpc + 1], f32)
        nc.vector.tensor_scalar(Bv[0:1, 0:bpc], pselq[:],
                                scalar1=scv[0:1, 0:1], scalar2=None, op0=AL.mult)
        nc.vector.tensor_scalar(Bv[0:1, bpc:2 * bpc], pselp[:],
                                scalar1=scv[0:1, 0:1], scalar2=None, op0=AL.mult)
        nc.vector.tensor_copy(Bv[0:1, 2 * bpc:2 * bpc + 1], scv[:])
        poffb = psum.tile([128, 2 * bpc + 1], f32, tag="ps")
        nc.tensor.matmul(poffb[:], ones_bc[:], Bv[:], start=True, stop=True)
        offb = keep.tile([128, 2 * bpc + 1], f32)
        nc.vector.tensor_copy(offb[:], poffb[:])

        # ---- phase E: in-place transform + store ----
        for t in range(ntiles):
            bl, h = t // 2, t % 2
            col = h * bpc + bl
            xt = xts[t]
            for c in range(nchunks):
                sl = slice(c * csz, (c + 1) * csz)
                nc.vector.tensor_scalar(xt[:, sl], xt[:, sl],
                                        scalar1=offb[:, 2 * bpc:2 * bpc + 1],
                                        scalar2=offb[:, col:col + 1],
                                        op0=AL.mult, op1=AL.add)
                eng = nc.gpsimd if (dma_mix and (t * nchunks + c) % 2) else nc.sync
                eng.dma_start(y[t][:, sl], xt[:, sl])

    nc.compile()
    return nc


def make_in_maps(inputs, ncores=NCORES, bpc=BPC, free=FREE):
    state = np.ascontiguousarray(np.asarray(inputs["state"], dtype=np.float32))
    dt = float(np.asarray(inputs["dt"]))
    nq = float(P * free)
    f = np.float32
    g = lambda k: np.ascontiguousarray(np.asarray(inputs[k], dtype=f))
    hW1, hW2, hW3, hW4 = g("hW1"), g("hW2"), g("hW3"), g("hW4")
    cW1 = g("cW1")
    common = {
        "w1a": np.ascontiguousarray(hW1[0:1, :]),
        "w1b": np.ascontiguousarray(hW1[1:2, :]),
        "w2": hW2, "w3": hW3,
        "b1": g("hb1").reshape(128, 1), "b2": g("hb2").reshape(128, 1),
        "b3": g("hb3").reshape(64, 1),
        "w4": hW4.reshape(64, 1), "w4n": np.ascontiguousarray(-hW4.reshape(64, 1)),
        "w1t": np.ascontiguousarray(hW1.T), "w2t": np.ascontiguousarray(hW2.T),
        "w3t": np.ascontiguousarray(hW3.T),
        "cw1a": np.ascontiguousarray(cW1[0:1, :]),
        "cw1b": np.ascontiguousarray(cW1[1:2, :]),
        "cw2": g("cW2"), "cw3": g("cW3"),
        "cb1": g("cb1").reshape(64, 1), "cb2": g("cb2").reshape(32, 1),
        "aux": np.array([[-0.5 * dt / nq, dt / nq]], dtype=f),
    }
    nb = ncores * bpc
    in_maps = []
    for i in range(ncores):
        selm = np.zeros((nb, bpc), dtype=f)
        for j in range(bpc):
            selm[i * bpc + j, j] = 1.0
        shard = np.ascontiguousarray(
            state[i * bpc:(i + 1) * bpc].reshape(2 * bpc, P, free))
        in_maps.append({"x": shard, "sel": selm, **common})
    return in_maps


def kernel(**inputs):
    from concourse.bass_utils import run_bass_kernel_spmd

    if "nc" not in _CACHE:
        _CACHE["nc"] = build_nc()
    nc = _CACHE["nc"]
    in_maps = make_in_maps(inputs)
    res = run_bass_kernel_spmd(nc, in_maps, list(range(NCORES)))
    out = np.concatenate(
        [res.results[i]["y"].reshape(BPC, CH, H, W) for i in range(NCORES)],
        axis=0)
    return out.astype(np.float32)



# revision 4
# speedup vs baseline: 1.4402x; 1.4402x over previous
"""Trainium2 Bass kernel for nn_HamiltonianDynamics.

Math: with q = state[:, :8], p = state[:, 8:], every MLP evaluation in the
reference operates on per-batch means of q/p. Adding a constant c to every
element of a [8,256,256] block shifts its mean by exactly c, so the whole
leapfrog chain (g1, g2, g3), the casimir correction and the global norm are
computable from just per-batch sums and sums of squares:

  out = (state + off[b, half]) * scale
  off_q[b] = dt*g2[b,1]/Nq,  off_p[b] = -0.5*dt*(g1[b,0]+g3[b,0])/Nq
  norm^2   = sum_b,h ( ssq[b,h] + 2*off[b,h]*sum[b,h] + Nq*off[b,h]^2 )
  scale    = 1 - 0.1*err/(norm+1e-10)

Data-parallel over batch: 4 batches per core. The data plane runs in bf16
(cast-on-load SWDGE DMA, bf16 store) which halves both DMA phases; the 2e-2
relative-error budget dwarfs bf16 rounding since out ~= state * (1 - 1e-11).

Per core: load shard as bf16 (resident in SBUF), per-tile sum via PE row-sum
matmuls + sum-of-squares via ScalarE Square/accum, local 4-batch MLP gradient
chain (batch on the free axis, features on partitions), then ONE tiny
AllGather of [perr, pnorm] partials, global scale, in-place transform, bf16
store. The leapfrog scale factors (+-dt terms), the casimir output reduction
and the -0.1/128 err normalizer are folded into host-side weight prep, and
the per-batch offset broadcast is precomputed during the collective wait.
"""

import numpy as np

NCORES = 8
B, CH, H, W = 32, 16, 256, 256
BPC = B // NCORES          # batches per core
NT = BPC * 2               # (batch, half) tiles per core
P = 128
FREE = (CH // 2) * H * W // P   # 4096
NQ = float(P * FREE)            # 524288

_CACHE: dict = {}


def build_nc(ncores=NCORES, bpc=BPC, free=FREE):
    import concourse.bass as bass
    import concourse.bacc as bacc
    import concourse.tile as tile
    import concourse.mybir as mybir
    from contextlib import ExitStack

    f32 = mybir.dt.float32
    bf16 = mybir.dt.bfloat16
    AL = mybir.AluOpType
    AF = mybir.ActivationFunctionType
    AX = mybir.AxisListType

    nt = bpc * 2
    nb = bpc
    nq = float(P * free)
    NSUB = 8                 # row-sum sub-chunks per tile (PE matmul width)
    SUB = free // NSUB       # 512
    LCH = 4                  # last tile is split for a short stats tail
    LSZ = free // LCH

    nc = bacc.Bacc("TRN2", target_bir_lowering=False, debug=False,
                   num_devices=ncores)

    def din(name, shape):
        return nc.dram_tensor(name, shape, f32, kind="ExternalInput").ap()

    x = din("x", [nt, P, free])
    w1a = din("w1a", [1, 128]);  w1b = din("w1b", [1, 128])
    b1 = din("b1", [128, 1])
    w2 = din("w2", [128, 128]);  b2 = din("b2", [128, 1])
    w3 = din("w3", [128, 64]);   b3 = din("b3", [64, 1])
    w4 = din("w4", [64, 1]);     w4n = din("w4n", [64, 1])
    w1tq = din("w1tq", [128, 1])   # hW1.T[:,0] * (-0.5*dt/Nq)
    w1tp = din("w1tp", [128, 1])   # hW1.T[:,1] * (dt/Nq)
    w2t = din("w2t", [128, 128]);  w3t = din("w3t", [64, 128])
    cw1a = din("cw1a", [1, 64]); cw1b = din("cw1b", [1, 64])
    cb1 = din("cb1", [64, 1])
    cw2 = din("cw2", [64, 32]);  cb2 = din("cb2", [32, 1])
    cw3s = din("cw3s", [32, 1])    # cW3 @ ones4 * (-0.1/128)
    y = nc.dram_tensor("y", [nt, P, free], bf16, kind="ExternalOutput").ap()

    with tile.TileContext(nc) as tc, ExitStack() as ctx:
        xpool = ctx.enter_context(tc.tile_pool(name="xp", bufs=1))
        wpool = ctx.enter_context(tc.tile_pool(name="wp", bufs=1))
        scr = ctx.enter_context(tc.tile_pool(name="scr", bufs=2))
        ch = ctx.enter_context(tc.tile_pool(name="ch", bufs=2))
        keep = ctx.enter_context(tc.tile_pool(name="keep", bufs=1))
        rpsum = ctx.enter_context(tc.tile_pool(name="rps", bufs=2, space="PSUM"))
        spsum = ctx.enter_context(tc.tile_pool(name="sps", bufs=1, space="PSUM"))
        cpsum = ctx.enter_context(tc.tile_pool(name="cps", bufs=1, space="PSUM"))
        psum = ctx.enter_context(tc.tile_pool(name="ps", bufs=4, space="PSUM"))
        dram = ctx.enter_context(tc.tile_pool(name="dr", bufs=1, space="DRAM"))

        ones_b = wpool.tile([128, 1], bf16)     # lhsT for bf16 row sums
        nc.vector.memset(ones_b[:], 1.0)
        ones_f = wpool.tile([128, 1], f32)      # lhsT for f32 partition sums
        nc.vector.memset(ones_f[:], 1.0)
        ones_bc = wpool.tile([1, 128], f32)     # lhsT for partition broadcast
        nc.vector.memset(ones_bc[:], 1.0)

        # ---- weights via HWDGE (sync queue idles during the SWDGE load) ----
        def wload(ap, shape):
            t = wpool.tile(shape, f32, tag=ap.tensor.name)
            nc.sync.dma_start(t[:], ap)
            return t

        w1a_sb = wload(w1a, [1, 128]); w1b_sb = wload(w1b, [1, 128])
        w2_sb = wload(w2, [128, 128]); w3_sb = wload(w3, [128, 64])
        b1_sb = wload(b1, [128, 1]); b2_sb = wload(b2, [128, 1])
        b3_sb = wload(b3, [64, 1])
        w4_sb = wload(w4, [64, 1]); w4n_sb = wload(w4n, [64, 1])
        w1tq_sb = wload(w1tq, [128, 1]); w1tp_sb = wload(w1tp, [128, 1])
        w2t_sb = wload(w2t, [128, 128]); w3t_sb = wload(w3t, [64, 128])
        cw1a_sb = wload(cw1a, [1, 64]); cw1b_sb = wload(cw1b, [1, 64])
        cb1_sb = wload(cb1, [64, 1])
        cw2_sb = wload(cw2, [64, 32]); cb2_sb = wload(cb2, [32, 1])
        cw3s_sb = wload(cw3s, [32, 1])

        # ---- phase A: cast-load shard to bf16, per-tile sum and sumsq ----
        S = keep.tile([1, nt], f32)             # per-tile sums
        PS = spsum.tile([1, nt], f32, tag="ssq")  # per-tile sum-of-squares
        xts = []
        for t in range(nt):
            xt = xpool.tile([P, free], bf16, tag=f"x{t}")
            if t < nt - 1:
                nc.gpsimd.dma_start(xt[:], x[t])       # f32 -> bf16 cast DMA
            else:
                for c in range(LCH):
                    sl = slice(c * LSZ, (c + 1) * LSZ)
                    nc.gpsimd.dma_start(xt[:, sl], x[t][:, sl])
            xts.append(xt)
            # row-sum: ones^T @ x accumulated over sub-chunks -> [1, SUB]
            rs = rpsum.tile([1, SUB], f32, tag="rs")
            for k in range(NSUB):
                nc.tensor.matmul(rs[:], ones_b[:], xt[:, k * SUB:(k + 1) * SUB],
                                 start=(k == 0), stop=(k == NSUB - 1))
            nc.vector.tensor_reduce(S[0:1, t:t + 1], rs[:], axis=AX.X, op=AL.add)
            # sum of squares via ScalarE Square + free-axis accumulate
            sq = scr.tile([P, free], bf16, tag="sq")
            if t < nt - 1:
                sst = keep.tile([P, 1], f32, tag=f"sst{t}")
                nc.scalar.activation(sq[:], xt[:], AF.Square, accum_out=sst[:])
                nc.tensor.matmul(PS[0:1, t:t + 1], ones_f[:], sst[:],
                                 start=True, stop=True)
            else:
                for c in range(LCH):
                    sl = slice(c * LSZ, (c + 1) * LSZ)
                    sstc = keep.tile([P, 1], f32, tag=f"sst{t}_{c}")
                    nc.scalar.activation(sq[:, sl], xt[:, sl], AF.Square,
                                         accum_out=sstc[:])
                    nc.tensor.matmul(PS[0:1, t:t + 1], ones_f[:], sstc[:],
                                     start=(c == 0), stop=(c == LCH - 1))

        # ---- phase B: local 4-batch gradient chain ----
        mq = keep.tile([1, nb], f32)
        nc.vector.tensor_scalar(mq[:], S[0:1, 0:nt:2], scalar1=1.0 / nq,
                                scalar2=None, op0=AL.mult)
        mp = keep.tile([1, nb], f32)
        nc.vector.tensor_scalar(mp[:], S[0:1, 1:nt:2], scalar1=1.0 / nq,
                                scalar2=None, op0=AL.mult)

        def gH(mq_ap, mp_ap, wsel, tag):
            """d(sum ham)/d(input col wsel), pre-scaled: [1,nb] psum."""
            p1 = psum.tile([128, nb], f32, tag="ps")
            nc.tensor.matmul(p1[:], w1a_sb[:], mq_ap, start=True, stop=False)
            nc.tensor.matmul(p1[:], w1b_sb[:], mp_ap, start=False, stop=True)
            h12 = ch.tile([128, 2 * nb], f32, tag=f"h12{tag}")
            nc.scalar.activation(h12[:, 0:nb], p1[:], AF.Tanh, bias=b1_sb[:])
            p2 = psum.tile([128, nb], f32, tag="ps")
            nc.tensor.matmul(p2[:], w2_sb[:], h12[:, 0:nb], start=True, stop=True)
            nc.scalar.activation(h12[:, nb:2 * nb], p2[:], AF.Tanh, bias=b2_sb[:])
            p3 = psum.tile([64, nb], f32, tag="ps")
            nc.tensor.matmul(p3[:], w3_sb[:], h12[:, nb:2 * nb],
                             start=True, stop=True)
            h3 = ch.tile([64, nb], f32, tag=f"h3{tag}")
            nc.scalar.activation(h3[:], p3[:], AF.Tanh, bias=b3_sb[:])
            # d3 = (1 - h3^2) * W4  ==  (h3^2) * (-W4) + W4
            d3 = ch.tile([64, nb], f32, tag=f"d3{tag}")
            nc.vector.tensor_tensor(d3[:], h3[:], h3[:], op=AL.mult)
            nc.vector.tensor_scalar(d3[:], d3[:], scalar1=w4n_sb[:],
                                    scalar2=w4_sb[:], op0=AL.mult, op1=AL.add)
            # 1 - h^2 for both hidden layers in one pass
            t12 = ch.tile([128, 2 * nb], f32, tag=f"t12{tag}")
            nc.vector.tensor_tensor(t12[:], h12[:], h12[:], op=AL.mult)
            nc.vector.tensor_scalar(t12[:], t12[:], scalar1=-1.0, scalar2=1.0,
                                    op0=AL.mult, op1=AL.add)
            pd2 = psum.tile([128, nb], f32, tag="ps")
            nc.tensor.matmul(pd2[:], w3t_sb[:], d3[:], start=True, stop=True)
            d2 = ch.tile([128, nb], f32, tag=f"d2{tag}")
            nc.vector.tensor_tensor(d2[:], t12[:, nb:2 * nb], pd2[:], op=AL.mult)
            pd1 = psum.tile([128, nb], f32, tag="ps")
            nc.tensor.matmul(pd1[:], w2t_sb[:], d2[:], start=True, stop=True)
            d1 = ch.tile([128, nb], f32, tag=f"d1{tag}")
            nc.vector.tensor_tensor(d1[:], t12[:, 0:nb], pd1[:], op=AL.mult)
            pg = psum.tile([1, nb], f32, tag="ps")
            nc.tensor.matmul(pg[:], wsel, d1[:], start=True, stop=True)
            return pg

        def cas_h2(mq_ap, mp_ap, tag, q1=None):
            """second hidden layer of casimir MLP -> [32,nb] sbuf."""
            if q1 is None:
                q1 = psum.tile([64, nb], f32, tag="ps")
                nc.tensor.matmul(q1[:], cw1a_sb[:], mq_ap, start=True, stop=False)
                nc.tensor.matmul(q1[:], cw1b_sb[:], mp_ap, start=False, stop=True)
            else:
                nc.tensor.matmul(q1[:], cw1b_sb[:], mp_ap, start=False, stop=True)
            g1 = ch.tile([64, nb], f32, tag=f"cg1{tag}")
            nc.scalar.activation(g1[:], q1[:], AF.Tanh, bias=cb1_sb[:])
            q2 = psum.tile([32, nb], f32, tag="ps")
            nc.tensor.matmul(q2[:], cw2_sb[:], g1[:], start=True, stop=True)
            g2 = ch.tile([32, nb], f32, tag=f"cg2{tag}")
            nc.scalar.activation(g2[:], q2[:], AF.Tanh, bias=cb2_sb[:])
            return g2

        # g1 (pre-scaled: o1 = -0.5*dt/Nq * dH/dq at (mq, mp))
        pg1 = gH(mq[:], mp[:], w1tq_sb[:], "1")
        o1s = keep.tile([1, nb], f32)
        nc.vector.tensor_copy(o1s[:], pg1[:])
        mp2 = keep.tile([1, nb], f32)
        nc.vector.tensor_tensor(mp2[:], mp[:], pg1[:], op=AL.add)
        # casimir old (overlaps the chain; only needs mq, mp)
        g2o = cas_h2(mq[:], mp[:], "o")
        # g2 (pre-scaled: offq = dt/Nq * dH/dp at (mq, mp2))
        pg2 = gH(mq[:], mp2[:], w1tp_sb[:], "2")
        offqs = keep.tile([1, nb], f32)
        nc.vector.tensor_copy(offqs[:], pg2[:])
        mq3 = keep.tile([1, nb], f32)
        nc.vector.tensor_tensor(mq3[:], mq[:], pg2[:], op=AL.add)
        # casimir-new layer-1 q-part can start during gH3
        q1n = cpsum.tile([64, nb], f32, tag="q1n")
        nc.tensor.matmul(q1n[:], cw1a_sb[:], mq3[:], start=True, stop=False)
        # g3 (pre-scaled: o3 = -0.5*dt/Nq * dH/dq at (mq3, mp2))
        pg3 = gH(mq3[:], mp2[:], w1tq_sb[:], "3")
        offps = keep.tile([1, nb], f32)
        nc.vector.tensor_tensor(offps[:], o1s[:], pg3[:], op=AL.add)
        mpn = keep.tile([1, nb], f32)
        nc.vector.tensor_tensor(mpn[:], mp[:], offps[:], op=AL.add)

        # ---- phase C: local partials (perr, pnorm), AllGather, scale ----
        cc = keep.tile([1, 2], f32)
        # perr: (-0.1/128) * sum_j,b (cas_new - cas_old)
        g2n = cas_h2(mq3[:], mpn[:], "n", q1=q1n)
        dh = ch.tile([32, nb], f32, tag="dh")
        nc.vector.tensor_tensor(dh[:], g2n[:], g2o[:], op=AL.subtract)
        pe_ = psum.tile([1, nb], f32, tag="ps")
        nc.tensor.matmul(pe_[:], cw3s_sb[:], dh[:], start=True, stop=True)
        nc.vector.tensor_reduce(cc[0:1, 0:1], pe_[:], axis=AX.X, op=AL.add)
        # pnorm: sum_b,h ssq + 2*off*sum + Nq*off^2  (sums precomputed)
        PSs = keep.tile([1, nt], f32)
        nc.vector.tensor_copy(PSs[:], PS[:])
        ssqsum = keep.tile([1, nb], f32)
        nc.vector.tensor_tensor(ssqsum[:], PSs[0:1, 0:nt:2], PSs[0:1, 1:nt:2],
                                op=AL.add)
        s2q = keep.tile([1, nb], f32)
        nc.vector.tensor_scalar(s2q[:], S[0:1, 0:nt:2], scalar1=2.0,
                                scalar2=None, op0=AL.mult)
        s2p = keep.tile([1, nb], f32)
        nc.vector.tensor_scalar(s2p[:], S[0:1, 1:nt:2], scalar1=2.0,
                                scalar2=None, op0=AL.mult)
        aq = keep.tile([1, nb], f32)
        nc.vector.scalar_tensor_tensor(aq[:], offqs[:], nq, s2q[:],
                                       op0=AL.mult, op1=AL.add)
        uq = keep.tile([1, nb], f32)
        nc.vector.tensor_tensor(uq[:], aq[:], offqs[:], op=AL.mult)
        ap_ = keep.tile([1, nb], f32)
        nc.vector.scalar_tensor_tensor(ap_[:], offps[:], nq, s2p[:],
                                       op0=AL.mult, op1=AL.add)
        up = keep.tile([1, nb], f32)
        nc.vector.tensor_tensor(up[:], ap_[:], offps[:], op=AL.mult)
        n2 = keep.tile([1, nb], f32)
        nc.vector.tensor_tensor(n2[:], uq[:], up[:], op=AL.add)
        nc.vector.tensor_tensor(n2[:], n2[:], ssqsum[:], op=AL.add)
        nc.vector.tensor_reduce(cc[0:1, 1:2], n2[:], axis=AX.X, op=AL.add)

        cc_in = dram.tile([1, 2], f32)
        cc_out = dram.tile([ncores, 2], f32)
        nc.sync.dma_start(cc_in[:], cc[:])
        nc.gpsimd.collective_compute(
            "AllGather", AL.bypass,
            replica_groups=[list(range(ncores))],
            ins=[cc_in[:].opt()], outs=[cc_out[:].opt()])

        # hidden under the collective: preload the rsqrt activation table and
        # broadcast the (unscaled) per-tile offsets across partitions
        dum = keep.tile([1, 1], f32)
        nc.scalar.activation(dum[:], cc[0:1, 1:2], AF.Abs_reciprocal_sqrt)
        Bv = keep.tile([1, nt], f32)
        nc.vector.tensor_copy(Bv[0:1, 0:nt:2], offqs[:])
        nc.vector.tensor_copy(Bv[0:1, 1:nt:2], offps[:])
        obp = psum.tile([128, nt], f32, tag="ps")
        nc.tensor.matmul(obp[:], ones_bc[:], Bv[:], start=True, stop=True)
        offb = keep.tile([128, nt], f32)
        nc.vector.tensor_copy(offb[:], obp[:])

        # ---- phase D: global scale ----
        G = keep.tile([1, 2 * ncores], f32)
        nc.sync.dma_start(G[:], cc_out[:, :])
        perr_t = keep.tile([1, 1], f32)
        nc.vector.tensor_reduce(perr_t[:], G[0:1, 0:2 * ncores:2],
                                axis=AX.X, op=AL.add)
        pnorm_t = keep.tile([1, 1], f32)
        nc.vector.tensor_reduce(pnorm_t[:], G[0:1, 1:2 * ncores:2],
                                axis=AX.X, op=AL.add)
        r = keep.tile([1, 1], f32)
        nc.scalar.activation(r[:], pnorm_t[:], AF.Abs_reciprocal_sqrt)
        sc = keep.tile([1, 1], f32)
        nc.vector.tensor_tensor(sc[:], r[:], perr_t[:], op=AL.mult)
        nc.vector.tensor_scalar(sc[:], sc[:], scalar1=1.0, scalar2=None,
                                op0=AL.add)
        bp = psum.tile([128, 1], f32, tag="ps")
        nc.tensor.matmul(bp[:], ones_bc[:], sc[:], start=True, stop=True)
        scb = keep.tile([128, 1], f32)
        nc.vector.tensor_copy(scb[:], bp[:])

        # ---- phase E: in-place transform + bf16 store ----
        for t in range(nt):
            xt = xts[t]
            nc.vector.tensor_scalar(xt[:], xt[:],
                                    scalar1=offb[:, t:t + 1],
                                    scalar2=scb[:, 0:1],
                                    op0=AL.add, op1=AL.mult)
            nc.sync.dma_start(y[t], xt[:])

    nc.compile()
    return nc


def make_in_maps(inputs, ncores=NCORES, bpc=BPC, free=FREE):
    state = np.asarray(inputs["state"], dtype=np.float32)
    dt = float(np.asarray(inputs["dt"]))
    nq = float(P * free)
    f = np.float32
    asc = np.ascontiguousarray
    g = lambda k: asc(np.asarray(inputs[k], dtype=f))
    hW1, hW2, hW3, hW4 = g("hW1"), g("hW2"), g("hW3"), g("hW4")
    cW1, cW3 = g("cW1"), g("cW3")
    w1t = hW1.T
    common = {
        "w1a": asc(hW1[0:1, :]), "w1b": asc(hW1[1:2, :]),
        "b1": g("hb1").reshape(128, 1),
        "w2": hW2, "b2": g("hb2").reshape(128, 1),
        "w3": hW3, "b3": g("hb3").reshape(64, 1),
        "w4": hW4.reshape(64, 1), "w4n": asc(-hW4.reshape(64, 1)),
        "w1tq": asc(w1t[:, 0:1] * f(-0.5 * dt / nq)),
        "w1tp": asc(w1t[:, 1:2] * f(dt / nq)),
        "w2t": asc(hW2.T), "w3t": asc(hW3.T),
        "cw1a": asc(cW1[0:1, :]), "cw1b": asc(cW1[1:2, :]),
        "cb1": g("cb1").reshape(64, 1),
        "cw2": g("cW2"), "cb2": g("cb2").reshape(32, 1),
        "cw3s": asc(cW3.sum(axis=1, keepdims=True) * f(-0.1 / (B * 4.0))),
    }
    in_maps = []
    for i in range(ncores):
        shard = asc(state[i * bpc:(i + 1) * bpc].reshape(2 * bpc, P, free))
        in_maps.append({"x": shard, **common})
    return in_maps


def kernel(**inputs):
    from concourse.bass_utils import run_bass_kernel_spmd

    if "nc" not in _CACHE:
        _CACHE["nc"] = build_nc()
    nc = _CACHE["nc"]
    in_maps = make_in_maps(inputs)
    res = run_bass_kernel_spmd(nc, in_maps, list(range(NCORES)))
    out = np.concatenate(
        [np.asarray(res.results[i]["y"]).astype(np.float32)
         .reshape(BPC, CH, H, W) for i in range(NCORES)],
        axis=0)
    return out


# revision 31
# speedup vs baseline: 1.6093x; 1.1174x over previous
"""Trainium2 Bass kernel for nn_HamiltonianDynamics.

Math: with q = state[:, :8], p = state[:, 8:], every MLP evaluation in the
reference operates on per-batch means of q/p. Adding a constant c to every
element of a [8,256,256] block shifts its mean by exactly c, so the whole
leapfrog chain (g1, g2, g3), the casimir correction and the global norm are
computable from just per-batch sums and sums of squares:

  out = (state + off[b, half]) * scale
  off_q[b] = dt*g2[b,1]/Nq,  off_p[b] = -0.5*dt*(g1[b,0]+g3[b,0])/Nq
  norm^2   = sum_b,h ( ssq[b,h] + 2*off[b,h]*sum[b,h] + Nq*off[b,h]^2 )
  scale    = 1 - 0.1*err/(norm+1e-10)

Data-parallel over batch: 4 batches per core. The data plane runs in bf16
(cast-on-load SWDGE DMA, bf16 store) which halves both DMA phases; the 2e-2
relative-error budget dwarfs bf16 rounding since out ~= state * (1 - 1e-11).

Per core: load shard as bf16 (resident in SBUF); per-tile sums via a DVE
copy-with-accumulate (4x mode), per-tile sum-of-squares via accumulated
[128,128] x^T x self-matmuls on the otherwise-idle tensor engine with the
trace extracted by a diag-mask scalar_tensor_tensor; local 4-batch MLP
gradient chain (batch on the free axis, features on partitions); ONE tiny
AllGather of [perr, pnorm] partials; global scale; in-place transform and
bf16 store. The leapfrog dt factors, the 1/Nq mean scaling, W4 (into the
backward w3w4 weights), the casimir output reduction and the -0.1/128 err
normalizer are all folded into host-side weight prep; the leapfrog mean
updates ride as extra accumulated K=1 matmuls in each layer-1 group; the
per-batch offset broadcast and the rsqrt activation-table load are hidden
under the collective wait.
"""

import numpy as np

NCORES = 8
B, CH, H, W = 32, 16, 256, 256
BPC = B // NCORES          # batches per core
NT = BPC * 2               # (batch, half) tiles per core
P = 128
FREE = (CH // 2) * H * W // P   # 4096
NQ = float(P * FREE)            # 524288
WP_COLS = 617                   # packed matrix-weights block width
WR_COLS = 768                   # packed row-vector weights

_CACHE: dict = {}


def build_nc(ncores=NCORES, bpc=BPC, free=FREE):
    import concourse.bass as bass
    import concourse.bacc as bacc
    import concourse.tile as tile
    import concourse.mybir as mybir
    from contextlib import ExitStack

    f32 = mybir.dt.float32
    bf16 = mybir.dt.bfloat16
    AL = mybir.AluOpType
    AF = mybir.ActivationFunctionType
    AX = mybir.AxisListType

    nt = bpc * 2
    nb = bpc
    nq = float(P * free)
    LCH = 4                  # last tile is split for a short stats tail
    LSZ = free // LCH
    NBK = free // 128        # x^T x self-matmul blocks per tile

    nc = bacc.Bacc("TRN2", target_bir_lowering=False, debug=False,
                   num_devices=ncores)

    def din(name, shape):
        return nc.dram_tensor(name, shape, f32, kind="ExternalInput").ap()

    x = din("x", [nt, P, free])
    wp = din("wp", [128, WP_COLS])   # matrix weights packed into one block
    wr = din("wr", [1, WR_COLS])     # row-vector weights (lhsT K=1 rows)
    y = nc.dram_tensor("y", [nt, P, free], bf16, kind="ExternalOutput").ap()

    with tile.TileContext(nc) as tc, ExitStack() as ctx:
        xpool = ctx.enter_context(tc.tile_pool(name="xp", bufs=1))
        wpool = ctx.enter_context(tc.tile_pool(name="wp", bufs=1))
        scr = ctx.enter_context(tc.tile_pool(name="scr", bufs=2))
        ch = ctx.enter_context(tc.tile_pool(name="ch", bufs=2))
        keep = ctx.enter_context(tc.tile_pool(name="keep", bufs=1))
        spsum = ctx.enter_context(tc.tile_pool(name="sps", bufs=1, space="PSUM"))
        qpsum = ctx.enter_context(tc.tile_pool(name="qps", bufs=2, space="PSUM"))
        psum = ctx.enter_context(tc.tile_pool(name="ps", bufs=4, space="PSUM"))
        dram = ctx.enter_context(tc.tile_pool(name="dr", bufs=1, space="DRAM"))

        ones_f = wpool.tile([128, 1], f32)      # lhsT for f32 partition sums
        nc.vector.memset(ones_f[:], 1.0)
        ones_bc = wpool.tile([1, 128], f32)     # lhsT for partition broadcast
        nc.vector.memset(ones_bc[:], 1.0)
        # preload the tanh activation table off the critical path
        dumt = keep.tile([1, 1], f32)
        nc.scalar.activation(dumt[:], ones_f[0:1, 0:1], AF.Tanh)

        # ---- weights: two packed DMAs on the HWDGE ring, slice views ----
        Wp = wpool.tile([128, WP_COLS], f32, tag="wp")
        nc.sync.dma_start(Wp[:], wp)
        Wr = wpool.tile([1, WR_COLS], f32, tag="wr")
        nc.sync.dma_start(Wr[:], wr)
        w2_sb = Wp[:, 0:128];      w2t_sb = Wp[:, 128:256]
        w3_sb = Wp[:, 256:320];    w3w4_sb = Wp[0:64, 320:448]
        b1_sb = Wp[:, 448:449];    b2_sb = Wp[:, 449:450]
        w1tq_sb = Wp[:, 450:451];  w1tp_sb = Wp[:, 451:452]
        b3_sb = Wp[0:64, 452:453]; cb1_sb = Wp[0:64, 453:454]
        cw2_sb = Wp[0:64, 454:486]; cb2_sb = Wp[0:32, 486:487]
        cw3s_sb = Wp[0:32, 487:488]
        cw3sn_sb = Wp[0:32, 488:489]
        I128 = Wp[:, 489:617]
        w1a_sb = Wr[0:1, 0:128];   w1b_sb = Wr[0:1, 128:256]
        cw1a_sb = Wr[0:1, 256:320]; cw1b_sb = Wr[0:1, 320:384]
        w1an_sb = Wr[0:1, 384:512]; w1bn_sb = Wr[0:1, 512:640]
        cw1an_sb = Wr[0:1, 640:704]; cw1bn_sb = Wr[0:1, 704:768]

        # ---- phase A: cast-load shard to bf16, per-tile sum and sumsq ----
        # sums: in-place copy with free-axis accumulate (DVE 4x mode).
        # sumsq: 32 accumulated [128,128] x^T x self-matmuls on the (idle)
        # tensor engine; trace extracted with a diag-mask scalar_tensor_tensor
        # against an identity block. ScalarE does nothing here, so the chain's
        # tanh ops are never queued behind stats work.
        PSa = spsum.tile([1, nt], f32, tag="psa")   # per-tile sums
        PSb = spsum.tile([1, nt], f32, tag="psb")   # per-tile sum of squares
        waste = scr.tile([P, free], bf16, tag="waste")  # sum-copy discard
        xts = []
        for t in range(nt):
            xt = xpool.tile([P, free], bf16, tag=f"x{t}")
            if t < nt - 1:
                chunks = [slice(0, free)]
            else:
                chunks = [slice(c * LSZ, (c + 1) * LSZ) for c in range(LCH)]
            for sl in chunks:
                nc.gpsimd.dma_start(xt[:, sl], x[t][:, sl])  # f32->bf16 cast
            xts.append(xt)
            ncol = len(chunks)
            for c, sl in enumerate(chunks):
                sc_ = keep.tile([P, 1], f32, tag=f"sc{t}_{c}")
                nc.vector.tensor_scalar(waste[:, sl], xt[:, sl], scalar1=1.0,
                                        scalar2=0.0, op0=AL.mult, op1=AL.add,
                                        accum_out=sc_[:])
                nc.tensor.matmul(PSa[0:1, t:t + 1], ones_f[:], sc_[:],
                                 start=(c == 0), stop=(c == ncol - 1))
            G2 = qpsum.tile([128, 128], f32, tag="g2")
            for k in range(NBK):
                sl2 = slice(128 * k, 128 * (k + 1))
                nc.tensor.matmul(G2[:], xt[:, sl2], xt[:, sl2],
                                 start=(k == 0), stop=(k == NBK - 1))
            dg = scr.tile([128, 128], f32, tag="dg")
            dcol = keep.tile([P, 1], f32, tag=f"dc{t}")
            nc.vector.scalar_tensor_tensor(dg[:], G2[:], 1.0, I128,
                                           op0=AL.mult, op1=AL.mult,
                                           accum_out=dcol[:])
            nc.tensor.matmul(PSb[0:1, t:t + 1], ones_f[:], dcol[:],
                             start=True, stop=True)
        # ---- phase B: local 4-batch gradient chain ----
        # raw per-tile sums to SBUF in one copy; the 1/Nq mean scaling is
        # folded into the _n layer-1 weight rows on the host
        Sqp = keep.tile([1, nt], f32)
        nc.vector.tensor_copy(Sqp[:], PSa[:])
        Sq = Sqp[0:1, 0:nt:2]
        Sp = Sqp[0:1, 1:nt:2]

        def gH(parts, wsel, tag):
            """d(sum ham)/d(input col wsel), pre-scaled: [1,nb] psum.

            parts: [(lhsT, rhs), ...] accumulated as the layer-1 input --
            folds the leapfrog mean updates into the matmul group. The three
            1-h^2 terms share one square+affine pass (W4 is folded into the
            host-prepped w3w4 backward weights)."""
            p1 = psum.tile([128, nb], f32, tag="ps")
            for i, (wl, rr) in enumerate(parts):
                nc.tensor.matmul(p1[:], wl, rr, start=(i == 0),
                                 stop=(i == len(parts) - 1))
            h123 = ch.tile([128, 3 * nb], f32, tag=f"h{tag}")
            nc.scalar.activation(h123[:, 0:nb], p1[:], AF.Tanh, bias=b1_sb)
            p2 = psum.tile([128, nb], f32, tag="ps")
            nc.tensor.matmul(p2[:], w2_sb, h123[:, 0:nb], start=True, stop=True)
            nc.scalar.activation(h123[:, nb:2 * nb], p2[:], AF.Tanh, bias=b2_sb)
            p3 = psum.tile([64, nb], f32, tag="ps")
            nc.tensor.matmul(p3[:], w3_sb, h123[:, nb:2 * nb],
                             start=True, stop=True)
            nc.scalar.activation(h123[0:64, 2 * nb:3 * nb], p3[:], AF.Tanh,
                                 bias=b3_sb)
            t123 = ch.tile([128, 3 * nb], f32, tag=f"t{tag}")
            nc.vector.tensor_tensor(t123[:], h123[:], h123[:], op=AL.mult)
            nc.vector.tensor_scalar(t123[:], t123[:], scalar1=-1.0, scalar2=1.0,
                                    op0=AL.mult, op1=AL.add)
            pd2 = psum.tile([128, nb], f32, tag="ps")
            nc.tensor.matmul(pd2[:], w3w4_sb, t123[0:64, 2 * nb:3 * nb],
                             start=True, stop=True)
            d2 = ch.tile([128, nb], f32, tag=f"d2{tag}")
            nc.vector.tensor_tensor(d2[:], t123[:, nb:2 * nb], pd2[:],
                                    op=AL.mult)
            pd1 = psum.tile([128, nb], f32, tag="ps")
            nc.tensor.matmul(pd1[:], w2t_sb, d2[:], start=True, stop=True)
            d1 = ch.tile([128, nb], f32, tag=f"d1{tag}")
            nc.vector.tensor_tensor(d1[:], t123[:, 0:nb], pd1[:], op=AL.mult)
            pg = psum.tile([1, nb], f32, tag="ps")
            nc.tensor.matmul(pg[:], wsel, d1[:], start=True, stop=True)
            return pg

        def cas_h2(parts, tag):
            """second hidden layer of casimir MLP -> [32,nb] sbuf."""
            q1 = psum.tile([64, nb], f32, tag="ps")
            for i, (wl, rr) in enumerate(parts):
                nc.tensor.matmul(q1[:], wl, rr, start=(i == 0),
                                 stop=(i == len(parts) - 1))
            g1 = ch.tile([64, nb], f32, tag=f"cg1{tag}")
            nc.scalar.activation(g1[:], q1[:], AF.Tanh, bias=cb1_sb)
            q2 = psum.tile([32, nb], f32, tag="ps")
            nc.tensor.matmul(q2[:], cw2_sb, g1[:], start=True, stop=True)
            g2 = ch.tile([32, nb], f32, tag=f"cg2{tag}")
            nc.scalar.activation(g2[:], q2[:], AF.Tanh, bias=cb2_sb)
            return g2

        # g1 (pre-scaled: o1 = -0.5*dt/Nq * dH/dq at (mq, mp))
        pg1 = gH([(w1an_sb, Sq), (w1bn_sb, Sp)], w1tq_sb, "1")
        o1s = keep.tile([1, nb], f32)
        nc.vector.tensor_copy(o1s[:], pg1[:])
        # casimir old (overlaps the chain; only needs the means)
        g2o = cas_h2([(cw1an_sb, Sq), (cw1bn_sb, Sp)], "o")
        # g2 = offq, evaluated at (mq, mp + o1): o1 folded into layer 1
        pg2 = gH([(w1an_sb, Sq), (w1bn_sb, Sp), (w1b_sb, o1s[:])],
                 w1tp_sb, "2")
        offqs = keep.tile([1, nb], f32)
        nc.vector.tensor_copy(offqs[:], pg2[:])
        # g3 = o3, evaluated at (mq + offq, mp + o1)
        pg3 = gH([(w1an_sb, Sq), (w1a_sb, offqs[:]), (w1bn_sb, Sp),
                  (w1b_sb, o1s[:])], w1tq_sb, "3")
        offps = keep.tile([1, nb], f32)
        nc.vector.tensor_tensor(offps[:], o1s[:], pg3[:], op=AL.add)

        # ---- phase C: local partials (perr, pnorm), AllGather, scale ----
        cc = keep.tile([1, 2], f32)
        # perr: (-0.1/128) * sum_j,b (cas_new - cas_old); cas_new evaluated
        # at (mq + offq, mp + offp) with the updates folded into layer 1.
        # Its layer-1 matmuls open early (during gH3) in a load-stats psum
        # bank, and the old/new difference is folded into one signed
        # accumulation group (cw3sn = -cw3s) instead of a subtract op.
        q1n = qpsum.tile([64, nb], f32, tag="g2")
        nc.tensor.matmul(q1n[:], cw1an_sb, Sq, start=True, stop=False)
        nc.tensor.matmul(q1n[:], cw1a_sb, offqs[:], start=False, stop=False)
        nc.tensor.matmul(q1n[:], cw1bn_sb, Sp, start=False, stop=False)
        nc.tensor.matmul(q1n[:], cw1b_sb, offps[:], start=False, stop=True)
        g1n = ch.tile([64, nb], f32, tag="cg1n")
        nc.scalar.activation(g1n[:], q1n[:], AF.Tanh, bias=cb1_sb)
        q2n = psum.tile([32, nb], f32, tag="ps")
        nc.tensor.matmul(q2n[:], cw2_sb, g1n[:], start=True, stop=True)
        g2n = ch.tile([32, nb], f32, tag="cg2n")
        nc.scalar.activation(g2n[:], q2n[:], AF.Tanh, bias=cb2_sb)
        pe_ = qpsum.tile([1, nb], f32, tag="g2")
        nc.tensor.matmul(pe_[:], cw3sn_sb, g2o[:], start=True, stop=False)
        nc.tensor.matmul(pe_[:], cw3s_sb, g2n[:], start=False, stop=True)
        nc.vector.tensor_reduce(cc[0:1, 0:1], pe_[:], axis=AX.X, op=AL.add)
        # pnorm: sum_b,h ssq + 2*off*sum + Nq*off^2  (sums precomputed)
        Qs = keep.tile([1, nt], f32)
        nc.vector.tensor_copy(Qs[:], PSb[:])
        ssqsum = keep.tile([1, nb], f32)
        nc.vector.tensor_tensor(ssqsum[:], Qs[0:1, 0:nt:2], Qs[0:1, 1:nt:2],
                                op=AL.add)
        s2q = keep.tile([1, nb], f32)
        nc.vector.tensor_scalar(s2q[:], Sq, scalar1=2.0,
                                scalar2=None, op0=AL.mult)
        s2p = keep.tile([1, nb], f32)
        nc.vector.tensor_scalar(s2p[:], Sp, scalar1=2.0,
                                scalar2=None, op0=AL.mult)
        aq = keep.tile([1, nb], f32)
        nc.vector.scalar_tensor_tensor(aq[:], offqs[:], nq, s2q[:],
                                       op0=AL.mult, op1=AL.add)
        uq = keep.tile([1, nb], f32)
        nc.vector.tensor_tensor(uq[:], aq[:], offqs[:], op=AL.mult)
        ap_ = keep.tile([1, nb], f32)
        nc.vector.scalar_tensor_tensor(ap_[:], offps[:], nq, s2p[:],
                                       op0=AL.mult, op1=AL.add)
        up = keep.tile([1, nb], f32)
        nc.vector.tensor_tensor(up[:], ap_[:], offps[:], op=AL.mult)
        n2 = keep.tile([1, nb], f32)
        nc.vector.tensor_tensor(n2[:], uq[:], up[:], op=AL.add)
        nc.vector.tensor_tensor(n2[:], n2[:], ssqsum[:], op=AL.add)
        nc.vector.tensor_reduce(cc[0:1, 1:2], n2[:], axis=AX.X, op=AL.add)

        cc_in = dram.tile([1, 2], f32)
        cc_out = dram.tile([ncores, 2], f32)
        nc.sync.dma_start(cc_in[:], cc[:])
        nc.gpsimd.collective_compute(
            "AllGather", AL.bypass,
            replica_groups=[list(range(ncores))],
            ins=[cc_in[:].opt()], outs=[cc_out[:].opt()])

        # hidden under the collective: preload the rsqrt activation table and
        # broadcast the (unscaled) per-tile offsets across partitions
        dum = keep.tile([1, 1], f32)
        nc.scalar.activation(dum[:], cc[0:1, 1:2], AF.Abs_reciprocal_sqrt)
        Bv = keep.tile([1, nt], f32)
        nc.vector.tensor_copy(Bv[0:1, 0:nt:2], offqs[:])
        nc.vector.tensor_copy(Bv[0:1, 1:nt:2], offps[:])
        obp = psum.tile([128, nt], f32, tag="ps")
        nc.tensor.matmul(obp[:], ones_bc[:], Bv[:], start=True, stop=True)
        offb = keep.tile([128, nt], f32)
        nc.vector.tensor_copy(offb[:], obp[:])

        # ---- phase D: global scale ----
        G = keep.tile([1, 2 * ncores], f32)
        nc.sync.dma_start(G[:], cc_out[:, :])
        perr_t = keep.tile([1, 1], f32)
        nc.vector.tensor_reduce(perr_t[:], G[0:1, 0:2 * ncores:2],
                                axis=AX.X, op=AL.add)
        pnorm_t = keep.tile([1, 1], f32)
        nc.vector.tensor_reduce(pnorm_t[:], G[0:1, 1:2 * ncores:2],
                                axis=AX.X, op=AL.add)
        r = keep.tile([1, 1], f32)
        nc.scalar.activation(r[:], pnorm_t[:], AF.Abs_reciprocal_sqrt)
        sc = keep.tile([1, 1], f32)
        nc.vector.scalar_tensor_tensor(sc[:], r[:], perr_t[:],
                                       ones_f[0:1, 0:1],
                                       op0=AL.mult, op1=AL.add)
        bp = psum.tile([128, 1], f32, tag="ps")
        nc.tensor.matmul(bp[:], ones_bc[:], sc[:], start=True, stop=True)
        scb = keep.tile([128, 1], f32)
        nc.vector.tensor_copy(scb[:], bp[:])

        # ---- phase E: in-place transform + bf16 store ----
        for t in range(nt):
            xt = xts[t]
            if t == 0:
                subs = [slice(c * LSZ, (c + 1) * LSZ) for c in range(LCH)]
            else:
                subs = [slice(0, free)]
            for sl in subs:
                nc.vector.tensor_scalar(xt[:, sl], xt[:, sl],
                                        scalar1=offb[:, t:t + 1],
                                        scalar2=scb[:, 0:1],
                                        op0=AL.add, op1=AL.mult)
                nc.sync.dma_start(y[t][:, sl], xt[:, sl])

    nc.compile()
    return nc


def make_in_maps(inputs, ncores=NCORES, bpc=BPC, free=FREE):
    state = np.asarray(inputs["state"], dtype=np.float32)
    dt = float(np.asarray(inputs["dt"]))
    nq = float(P * free)
    f = np.float32
    g = lambda k: np.asarray(inputs[k], dtype=f)
    hW1, hW2, hW3, hW4 = g("hW1"), g("hW2"), g("hW3"), g("hW4")
    cW1, cW3 = g("cW1"), g("cW3")
    w1t = hW1.T

    wp = np.zeros((128, WP_COLS), dtype=f)
    wp[:, 0:128] = hW2
    wp[:, 128:256] = hW2.T
    wp[:, 256:320] = hW3
    wp[0:64, 320:448] = hW3.T * hW4.reshape(64, 1)
    wp[:, 448] = g("hb1")
    wp[:, 449] = g("hb2")
    wp[:, 450] = w1t[:, 0] * f(-0.5 * dt / nq)
    wp[:, 451] = w1t[:, 1] * f(dt / nq)
    wp[0:64, 452] = g("hb3")
    wp[0:64, 453] = g("cb1")
    wp[0:64, 454:486] = g("cW2")
    wp[0:32, 486] = g("cb2")
    wp[0:32, 487] = cW3.sum(axis=1) * f(-0.1 / (B * 4.0))
    wp[0:32, 488] = cW3.sum(axis=1) * f(0.1 / (B * 4.0))
    wp[:, 489:617] = np.eye(128, dtype=f)

    wr = np.zeros((1, WR_COLS), dtype=f)
    wr[0, 0:128] = hW1[0, :]
    wr[0, 128:256] = hW1[1, :]
    wr[0, 256:320] = cW1[0, :]
    wr[0, 320:384] = cW1[1, :]
    wr[0, 384:512] = hW1[0, :] / f(NQ)
    wr[0, 512:640] = hW1[1, :] / f(NQ)
    wr[0, 640:704] = cW1[0, :] / f(NQ)
    wr[0, 704:768] = cW1[1, :] / f(NQ)

    in_maps = []
    for i in range(ncores):
        shard = np.ascontiguousarray(
            state[i * bpc:(i + 1) * bpc].reshape(2 * bpc, P, free))
        in_maps.append({"x": shard, "wp": wp, "wr": wr})
    return in_maps


def kernel(**inputs):
    from concourse.bass_utils import run_bass_kernel_spmd

    if "nc" not in _CACHE:
        _CACHE["nc"] = build_nc()
    nc = _CACHE["nc"]
    in_maps = make_in_maps(inputs)
    res = run_bass_kernel_spmd(nc, in_maps, list(range(NCORES)))
    out = np.concatenate(
        [np.asarray(res.results[i]["y"]).astype(np.float32)
         .reshape(BPC, CH, H, W) for i in range(NCORES)],
        axis=0)
    return out


# revision 33
# speedup vs baseline: 1.6254x; 1.0100x over previous
"""Trainium2 Bass kernel for nn_HamiltonianDynamics.

Math: with q = state[:, :8], p = state[:, 8:], every MLP evaluation in the
reference operates on per-batch means of q/p. Adding a constant c to every
element of a [8,256,256] block shifts its mean by exactly c, so the whole
leapfrog chain (g1, g2, g3), the casimir correction and the global norm are
computable from just per-batch sums and sums of squares:

  out = (state + off[b, half]) * scale
  off_q[b] = dt*g2[b,1]/Nq,  off_p[b] = -0.5*dt*(g1[b,0]+g3[b,0])/Nq
  norm^2   = sum_b,h ( ssq[b,h] + 2*off[b,h]*sum[b,h] + Nq*off[b,h]^2 )
  scale    = 1 - 0.1*err/(norm+1e-10)

Data-parallel over batch: 4 batches per core. The data plane runs in bf16
(cast-on-load SWDGE DMA, bf16 store) which halves both DMA phases; the 2e-2
relative-error budget dwarfs bf16 rounding since out ~= state * (1 - 1e-11).

Per core: load shard as bf16 (resident in SBUF); per-tile sums via a DVE
copy-with-accumulate (4x mode), per-tile sum-of-squares via accumulated
[128,128] x^T x self-matmuls on the otherwise-idle tensor engine with the
trace extracted by a diag-mask scalar_tensor_tensor; local 4-batch MLP
gradient chain (batch on the free axis, features on partitions); ONE tiny
AllGather of [perr, pnorm] partials; global scale; in-place transform and
bf16 store. The leapfrog dt factors, the 1/Nq mean scaling, W4 (into the
backward w3w4 weights), the casimir output reduction and the -0.1/128 err
normalizer are all folded into host-side weight prep; the leapfrog mean
updates ride as extra accumulated K=1 matmuls in each layer-1 group; the
per-batch offset broadcast and the rsqrt activation-table load are hidden
under the collective wait.
"""

import numpy as np

NCORES = 8
B, CH, H, W = 32, 16, 256, 256
BPC = B // NCORES          # batches per core
NT = BPC * 2               # (batch, half) tiles per core
P = 128
FREE = (CH // 2) * H * W // P   # 4096
NQ = float(P * FREE)            # 524288
WP_COLS = 1001                  # packed matrix-weights block width (bf16)
WR_COLS = 768                   # packed row-vector weights

_CACHE: dict = {}


def build_nc(ncores=NCORES, bpc=BPC, free=FREE):
    import concourse.bass as bass
    import concourse.bacc as bacc
    import concourse.tile as tile
    import concourse.mybir as mybir
    from contextlib import ExitStack

    f32 = mybir.dt.float32
    bf16 = mybir.dt.bfloat16
    AL = mybir.AluOpType
    AF = mybir.ActivationFunctionType
    AX = mybir.AxisListType

    nt = bpc * 2
    nb = bpc
    nq = float(P * free)
    LCH = 4                  # last tile is split for a short stats tail
    LSZ = free // LCH
    NBK = free // 128        # x^T x self-matmul blocks per tile

    nc = bacc.Bacc("TRN2", target_bir_lowering=False, debug=False,
                   num_devices=ncores)

    def din(name, shape):
        return nc.dram_tensor(name, shape, f32, kind="ExternalInput").ap()

    x = din("x", [nt, P, free])
    wp = nc.dram_tensor("wp", [128, WP_COLS], bf16,
                        kind="ExternalInput").ap()  # matrix weights (bf16)
    wr = din("wr", [1, WR_COLS])     # row-vector weights (lhsT K=1 rows)
    y = nc.dram_tensor("y", [nt, P, free], bf16, kind="ExternalOutput").ap()

    with tile.TileContext(nc) as tc, ExitStack() as ctx:
        xpool = ctx.enter_context(tc.tile_pool(name="xp", bufs=1))
        wpool = ctx.enter_context(tc.tile_pool(name="wp", bufs=1))
        scr = ctx.enter_context(tc.tile_pool(name="scr", bufs=2))
        ch = ctx.enter_context(tc.tile_pool(name="ch", bufs=2))
        keep = ctx.enter_context(tc.tile_pool(name="keep", bufs=1))
        spsum = ctx.enter_context(tc.tile_pool(name="sps", bufs=1, space="PSUM"))
        qpsum = ctx.enter_context(tc.tile_pool(name="qps", bufs=2, space="PSUM"))
        psum = ctx.enter_context(tc.tile_pool(name="ps", bufs=4, space="PSUM"))
        dram = ctx.enter_context(tc.tile_pool(name="dr", bufs=1, space="DRAM"))

        ones_f = wpool.tile([128, 1], f32)      # lhsT for f32 partition sums
        nc.vector.memset(ones_f[:], 1.0)
        ones_bc = wpool.tile([1, 128], f32)     # lhsT for partition broadcast
        nc.vector.memset(ones_bc[:], 1.0)
        # preload the tanh activation table off the critical path
        dumt = keep.tile([1, 1], f32)
        nc.scalar.activation(dumt[:], ones_f[0:1, 0:1], AF.Tanh)

        # ---- weights: bf16 packed DMA (halves the transfer ahead of the
        # x stream), cast once to f32 in SBUF; row vectors tiny f32 ----
        Wpb = wpool.tile([128, WP_COLS], bf16, tag="wpb")
        nc.sync.dma_start(Wpb[:], wp)
        Wr = wpool.tile([1, WR_COLS], f32, tag="wr")
        nc.sync.dma_start(Wr[:], wr)
        Wp = wpool.tile([128, WP_COLS], f32, tag="wp")
        nc.vector.tensor_copy(Wp[:], Wpb[:])
        w2_sb = Wp[:, 0:128];      w2t_sb = Wp[:, 128:256]
        w3_sb = Wp[:, 256:320];    w3w4_sb = Wp[0:64, 320:448]
        b1_sb = Wp[:, 448:449];    b2_sb = Wp[:, 449:450]
        w1tq_sb = Wp[:, 450:451];  w1tp_sb = Wp[:, 451:452]
        b3_sb = Wp[0:64, 452:453]; cb1_sb = Wp[0:64, 453:454]
        cw2_sb = Wp[0:64, 454:486]; cb2_sb = Wp[0:32, 486:487]
        cw3s_sb = Wp[0:32, 487:488]
        cw3sn_sb = Wp[0:32, 488:489]
        I128 = Wp[:, 489:617]
        Woa_sb = Wp[:, 617:745];   Wob_sb = Wp[:, 745:873]
        Woca_sb = Wp[:, 873:937];  Wocb_sb = Wp[:, 937:1001]
        w1a_sb = Wr[0:1, 0:128];   w1b_sb = Wr[0:1, 128:256]
        cw1a_sb = Wr[0:1, 256:320]; cw1b_sb = Wr[0:1, 320:384]
        w1an_sb = Wr[0:1, 384:512]; w1bn_sb = Wr[0:1, 512:640]
        cw1an_sb = Wr[0:1, 640:704]; cw1bn_sb = Wr[0:1, 704:768]

        # ---- phase A: cast-load shard to bf16, per-tile sum and sumsq ----
        # sums: in-place copy with free-axis accumulate (DVE 4x mode).
        # sumsq: 32 accumulated [128,128] x^T x self-matmuls on the (idle)
        # tensor engine; trace extracted with a diag-mask scalar_tensor_tensor
        # against an identity block. ScalarE does nothing here, so the chain's
        # tanh ops are never queued behind stats work.
        PSa = spsum.tile([1, nt], f32, tag="psa")   # per-tile sums
        PSb = spsum.tile([1, nt], f32, tag="psb")   # per-tile sum of squares
        waste = scr.tile([P, free], bf16, tag="waste")  # sum-copy discard
        SCq = keep.tile([P, nb], f32)   # per-partition sum partials, q tiles
        SCp = keep.tile([P, nb], f32)   # per-partition sum partials, p tiles
        SC7 = keep.tile([P, LCH], f32)  # last tile's per-chunk partials
        xts = []
        for t in range(nt):
            xt = xpool.tile([P, free], bf16, tag=f"x{t}")
            if t < nt - 1:
                chunks = [slice(0, free)]
            else:
                chunks = [slice(c * LSZ, (c + 1) * LSZ) for c in range(LCH)]
            for sl in chunks:
                nc.gpsimd.dma_start(xt[:, sl], x[t][:, sl])  # f32->bf16 cast
            xts.append(xt)
            ncol = len(chunks)
            for c, sl in enumerate(chunks):
                if t == nt - 1:
                    sc_ = SC7[:, c:c + 1]
                elif t % 2 == 0:
                    sc_ = SCq[:, t // 2:t // 2 + 1]
                else:
                    sc_ = SCp[:, t // 2:t // 2 + 1]
                nc.vector.tensor_scalar(waste[:, sl], xt[:, sl], scalar1=1.0,
                                        scalar2=0.0, op0=AL.mult, op1=AL.add,
                                        accum_out=sc_)
                nc.tensor.matmul(PSa[0:1, t:t + 1], ones_f[:], sc_,
                                 start=(c == 0), stop=(c == ncol - 1))
            G2 = qpsum.tile([128, 128], f32, tag="g2")
            for k in range(NBK):
                sl2 = slice(128 * k, 128 * (k + 1))
                nc.tensor.matmul(G2[:], xt[:, sl2], xt[:, sl2],
                                 start=(k == 0), stop=(k == NBK - 1))
            dg = scr.tile([128, 128], f32, tag="dg")
            dcol = keep.tile([P, 1], f32, tag=f"dc{t}")
            nc.vector.scalar_tensor_tensor(dg[:], G2[:], 1.0, I128,
                                           op0=AL.mult, op1=AL.mult,
                                           accum_out=dcol[:])
            nc.tensor.matmul(PSb[0:1, t:t + 1], ones_f[:], dcol[:],
                             start=True, stop=True)
        # ---- phase B: local 4-batch gradient chain ----
        # layer-1 consumes the raw [128,1] partials via host-built
        # outer-product weights (ones (x) w1row/Nq), so the chain does not
        # wait for the PSa partition-sum + SBUF copy. The scalar sums are
        # still materialized (off the critical path) for the pnorm terms.
        nc.vector.tensor_reduce(SCp[:, nb - 1:nb], SC7[:], axis=AX.X,
                                op=AL.add)
        Sqp = keep.tile([1, nt], f32)
        nc.vector.tensor_copy(Sqp[:], PSa[:])
        Sq = Sqp[0:1, 0:nt:2]
        Sp = Sqp[0:1, 1:nt:2]

        def gH(parts, wsel, tag):
            """d(sum ham)/d(input col wsel), pre-scaled: [1,nb] psum.

            parts: [(lhsT, rhs), ...] accumulated as the layer-1 input --
            folds the leapfrog mean updates into the matmul group. The three
            1-h^2 terms share one square+affine pass (W4 is folded into the
            host-prepped w3w4 backward weights)."""
            p1 = psum.tile([128, nb], f32, tag="ps")
            for i, (wl, rr) in enumerate(parts):
                nc.tensor.matmul(p1[:], wl, rr, start=(i == 0),
                                 stop=(i == len(parts) - 1))
            h123 = ch.tile([128, 3 * nb], f32, tag=f"h{tag}")
            nc.scalar.activation(h123[:, 0:nb], p1[:], AF.Tanh, bias=b1_sb)
            p2 = psum.tile([128, nb], f32, tag="ps")
            nc.tensor.matmul(p2[:], w2_sb, h123[:, 0:nb], start=True, stop=True)
            nc.scalar.activation(h123[:, nb:2 * nb], p2[:], AF.Tanh, bias=b2_sb)
            p3 = psum.tile([64, nb], f32, tag="ps")
            nc.tensor.matmul(p3[:], w3_sb, h123[:, nb:2 * nb],
                             start=True, stop=True)
            nc.scalar.activation(h123[0:64, 2 * nb:3 * nb], p3[:], AF.Tanh,
                                 bias=b3_sb)
            t123 = ch.tile([128, 3 * nb], f32, tag=f"t{tag}")
            nc.vector.tensor_tensor(t123[:], h123[:], h123[:], op=AL.mult)
            nc.vector.tensor_scalar(t123[:], t123[:], scalar1=-1.0, scalar2=1.0,
                                    op0=AL.mult, op1=AL.add)
            pd2 = psum.tile([128, nb], f32, tag="ps")
            nc.tensor.matmul(pd2[:], w3w4_sb, t123[0:64, 2 * nb:3 * nb],
                             start=True, stop=True)
            d2 = ch.tile([128, nb], f32, tag=f"d2{tag}")
            nc.vector.tensor_tensor(d2[:], t123[:, nb:2 * nb], pd2[:],
                                    op=AL.mult)
            pd1 = psum.tile([128, nb], f32, tag="ps")
            nc.tensor.matmul(pd1[:], w2t_sb, d2[:], start=True, stop=True)
            d1 = ch.tile([128, nb], f32, tag=f"d1{tag}")
            nc.vector.tensor_tensor(d1[:], t123[:, 0:nb], pd1[:], op=AL.mult)
            pg = psum.tile([1, nb], f32, tag="ps")
            nc.tensor.matmul(pg[:], wsel, d1[:], start=True, stop=True)
            return pg

        def cas_h2(parts, tag):
            """second hidden layer of casimir MLP -> [32,nb] sbuf."""
            q1 = psum.tile([64, nb], f32, tag="ps")
            for i, (wl, rr) in enumerate(parts):
                nc.tensor.matmul(q1[:], wl, rr, start=(i == 0),
                                 stop=(i == len(parts) - 1))
            g1 = ch.tile([64, nb], f32, tag=f"cg1{tag}")
            nc.scalar.activation(g1[:], q1[:], AF.Tanh, bias=cb1_sb)
            q2 = psum.tile([32, nb], f32, tag="ps")
            nc.tensor.matmul(q2[:], cw2_sb, g1[:], start=True, stop=True)
            g2 = ch.tile([32, nb], f32, tag=f"cg2{tag}")
            nc.scalar.activation(g2[:], q2[:], AF.Tanh, bias=cb2_sb)
            return g2

        # g1 (pre-scaled: o1 = -0.5*dt/Nq * dH/dq at (mq, mp))
        pg1 = gH([(Woa_sb, SCq[:]), (Wob_sb, SCp[:])], w1tq_sb, "1")
        o1s = keep.tile([1, nb], f32)
        nc.vector.tensor_copy(o1s[:], pg1[:])
        # casimir old (overlaps the chain; only needs the means)
        g2o = cas_h2([(Woca_sb, SCq[:]), (Wocb_sb, SCp[:])], "o")
        # g2 = offq, evaluated at (mq, mp + o1): o1 folded into layer 1
        pg2 = gH([(Woa_sb, SCq[:]), (Wob_sb, SCp[:]), (w1b_sb, o1s[:])],
                 w1tp_sb, "2")
        offqs = keep.tile([1, nb], f32)
        nc.vector.tensor_copy(offqs[:], pg2[:])
        # g3 = o3, evaluated at (mq + offq, mp + o1)
        pg3 = gH([(Woa_sb, SCq[:]), (w1a_sb, offqs[:]), (Wob_sb, SCp[:]),
                  (w1b_sb, o1s[:])], w1tq_sb, "3")
        offps = keep.tile([1, nb], f32)
        nc.vector.tensor_tensor(offps[:], o1s[:], pg3[:], op=AL.add)

        # ---- phase C: local partials (perr, pnorm), AllGather, scale ----
        cc = keep.tile([1, 2], f32)
        # perr: (-0.1/128) * sum_j,b (cas_new - cas_old); cas_new evaluated
        # at (mq + offq, mp + offp) with the updates folded into layer 1.
        # Its layer-1 matmuls open early (during gH3) in a load-stats psum
        # bank, and the old/new difference is folded into one signed
        # accumulation group (cw3sn = -cw3s) instead of a subtract op.
        q1n = qpsum.tile([64, nb], f32, tag="g2")
        nc.tensor.matmul(q1n[:], Woca_sb, SCq[:], start=True, stop=False)
        nc.tensor.matmul(q1n[:], cw1a_sb, offqs[:], start=False, stop=False)
        nc.tensor.matmul(q1n[:], Wocb_sb, SCp[:], start=False, stop=False)
        nc.tensor.matmul(q1n[:], cw1b_sb, offps[:], start=False, stop=True)
        g1n = ch.tile([64, nb], f32, tag="cg1n")
        nc.scalar.activation(g1n[:], q1n[:], AF.Tanh, bias=cb1_sb)
        q2n = psum.tile([32, nb], f32, tag="ps")
        nc.tensor.matmul(q2n[:], cw2_sb, g1n[:], start=True, stop=True)
        g2n = ch.tile([32, nb], f32, tag="cg2n")
        nc.scalar.activation(g2n[:], q2n[:], AF.Tanh, bias=cb2_sb)
        pe_ = qpsum.tile([1, nb], f32, tag="g2")
        nc.tensor.matmul(pe_[:], cw3sn_sb, g2o[:], start=True, stop=False)
        nc.tensor.matmul(pe_[:], cw3s_sb, g2n[:], start=False, stop=True)
        nc.vector.tensor_reduce(cc[0:1, 0:1], pe_[:], axis=AX.X, op=AL.add)
        # pnorm: sum_b,h ssq + 2*off*sum + Nq*off^2  (sums precomputed)
        Qs = keep.tile([1, nt], f32)
        nc.vector.tensor_copy(Qs[:], PSb[:])
        ssqsum = keep.tile([1, nb], f32)
        nc.vector.tensor_tensor(ssqsum[:], Qs[0:1, 0:nt:2], Qs[0:1, 1:nt:2],
                                op=AL.add)
        s2q = keep.tile([1, nb], f32)
        nc.vector.tensor_scalar(s2q[:], Sq, scalar1=2.0,
                                scalar2=None, op0=AL.mult)
        s2p = keep.tile([1, nb], f32)
        nc.vector.tensor_scalar(s2p[:], Sp, scalar1=2.0,
                                scalar2=None, op0=AL.mult)
        aq = keep.tile([1, nb], f32)
        nc.vector.scalar_tensor_tensor(aq[:], offqs[:], nq, s2q[:],
                                       op0=AL.mult, op1=AL.add)
        uq = keep.tile([1, nb], f32)
        nc.vector.tensor_tensor(uq[:], aq[:], offqs[:], op=AL.mult)
        ap_ = keep.tile([1, nb], f32)
        nc.vector.scalar_tensor_tensor(ap_[:], offps[:], nq, s2p[:],
                                       op0=AL.mult, op1=AL.add)
        up = keep.tile([1, nb], f32)
        nc.vector.tensor_tensor(up[:], ap_[:], offps[:], op=AL.mult)
        n2 = keep.tile([1, nb], f32)
        nc.vector.tensor_tensor(n2[:], uq[:], up[:], op=AL.add)
        nc.vector.tensor_tensor(n2[:], n2[:], ssqsum[:], op=AL.add)
        nc.vector.tensor_reduce(cc[0:1, 1:2], n2[:], axis=AX.X, op=AL.add)

        cc_in = dram.tile([1, 2], f32)
        cc_out = dram.tile([ncores, 2], f32)
        nc.sync.dma_start(cc_in[:], cc[:])
        nc.gpsimd.collective_compute(
            "AllGather", AL.bypass,
            replica_groups=[list(range(ncores))],
            ins=[cc_in[:].opt()], outs=[cc_out[:].opt()])

        # hidden under the collective: preload the rsqrt activation table and
        # broadcast the (unscaled) per-tile offsets across partitions
        dum = keep.tile([1, 1], f32)
        nc.scalar.activation(dum[:], cc[0:1, 1:2], AF.Abs_reciprocal_sqrt)
        Bv = keep.tile([1, nt], f32)
        nc.vector.tensor_copy(Bv[0:1, 0:nt:2], offqs[:])
        nc.vector.tensor_copy(Bv[0:1, 1:nt:2], offps[:])
        obp = psum.tile([128, nt], f32, tag="ps")
        nc.tensor.matmul(obp[:], ones_bc[:], Bv[:], start=True, stop=True)
        offb = keep.tile([128, nt], f32)
        nc.vector.tensor_copy(offb[:], obp[:])

        # ---- phase D: global scale ----
        G = keep.tile([1, 2 * ncores], f32)
        nc.sync.dma_start(G[:], cc_out[:, :])
        perr_t = keep.tile([1, 1], f32)
        nc.vector.tensor_reduce(perr_t[:], G[0:1, 0:2 * ncores:2],
                                axis=AX.X, op=AL.add)
        pnorm_t = keep.tile([1, 1], f32)
        nc.vector.tensor_reduce(pnorm_t[:], G[0:1, 1:2 * ncores:2],
                                axis=AX.X, op=AL.add)
        r = keep.tile([1, 1], f32)
        nc.scalar.activation(r[:], pnorm_t[:], AF.Abs_reciprocal_sqrt)
        sc = keep.tile([1, 1], f32)
        nc.vector.scalar_tensor_tensor(sc[:], r[:], perr_t[:],
                                       ones_f[0:1, 0:1],
                                       op0=AL.mult, op1=AL.add)
        bp = psum.tile([128, 1], f32, tag="ps")
        nc.tensor.matmul(bp[:], ones_bc[:], sc[:], start=True, stop=True)
        scb = keep.tile([128, 1], f32)
        nc.vector.tensor_copy(scb[:], bp[:])

        # ---- phase E: in-place transform + bf16 store ----
        for t in range(nt):
            xt = xts[t]
            if t == 0:
                subs = [slice(c * LSZ, (c + 1) * LSZ) for c in range(LCH)]
            else:
                subs = [slice(0, free)]
            for sl in subs:
                nc.vector.tensor_scalar(xt[:, sl], xt[:, sl],
                                        scalar1=offb[:, t:t + 1],
                                        scalar2=scb[:, 0:1],
                                        op0=AL.add, op1=AL.mult)
                nc.sync.dma_start(y[t][:, sl], xt[:, sl])

    nc.compile()
    return nc


def make_in_maps(inputs, ncores=NCORES, bpc=BPC, free=FREE):
    state = np.asarray(inputs["state"], dtype=np.float32)
    dt = float(np.asarray(inputs["dt"]))
    nq = float(P * free)
    f = np.float32
    g = lambda k: np.asarray(inputs[k], dtype=f)
    hW1, hW2, hW3, hW4 = g("hW1"), g("hW2"), g("hW3"), g("hW4")
    cW1, cW3 = g("cW1"), g("cW3")
    w1t = hW1.T

    wp = np.zeros((128, WP_COLS), dtype=f)
    wp[:, 0:128] = hW2
    wp[:, 128:256] = hW2.T
    wp[:, 256:320] = hW3
    wp[0:64, 320:448] = hW3.T * hW4.reshape(64, 1)
    wp[:, 448] = g("hb1")
    wp[:, 449] = g("hb2")
    wp[:, 450] = w1t[:, 0] * f(-0.5 * dt / nq)
    wp[:, 451] = w1t[:, 1] * f(dt / nq)
    wp[0:64, 452] = g("hb3")
    wp[0:64, 453] = g("cb1")
    wp[0:64, 454:486] = g("cW2")
    wp[0:32, 486] = g("cb2")
    wp[0:32, 487] = cW3.sum(axis=1) * f(-0.1 / (B * 4.0))
    wp[0:32, 488] = cW3.sum(axis=1) * f(0.1 / (B * 4.0))
    wp[:, 489:617] = np.eye(128, dtype=f)
    wp[:, 617:745] = np.tile(hW1[0, :] / f(NQ), (128, 1))
    wp[:, 745:873] = np.tile(hW1[1, :] / f(NQ), (128, 1))
    wp[:, 873:937] = np.tile(cW1[0, :] / f(NQ), (128, 1))
    wp[:, 937:1001] = np.tile(cW1[1, :] / f(NQ), (128, 1))

    wr = np.zeros((1, WR_COLS), dtype=f)
    wr[0, 0:128] = hW1[0, :]
    wr[0, 128:256] = hW1[1, :]
    wr[0, 256:320] = cW1[0, :]
    wr[0, 320:384] = cW1[1, :]
    wr[0, 384:512] = hW1[0, :] / f(NQ)
    wr[0, 512:640] = hW1[1, :] / f(NQ)
    wr[0, 640:704] = cW1[0, :] / f(NQ)
    wr[0, 704:768] = cW1[1, :] / f(NQ)

    import ml_dtypes
    wpb = wp.astype(ml_dtypes.bfloat16)

    in_maps = []
    for i in range(ncores):
        shard = np.ascontiguousarray(
            state[i * bpc:(i + 1) * bpc].reshape(2 * bpc, P, free))
        in_maps.append({"x": shard, "wp": wpb, "wr": wr})
    return in_maps


def kernel(**inputs):
    from concourse.bass_utils import run_bass_kernel_spmd

    if "nc" not in _CACHE:
        _CACHE["nc"] = build_nc()
    nc = _CACHE["nc"]
    in_maps = make_in_maps(inputs)
    res = run_bass_kernel_spmd(nc, in_maps, list(range(NCORES)))
    out = np.concatenate(
        [np.asarray(res.results[i]["y"]).astype(np.float32)
         .reshape(BPC, CH, H, W) for i in range(NCORES)],
        axis=0)
    return out
